# revision 2
# baseline (speedup 1.0000x reference)
"""
Trainium2 Bass kernel for nn_BMM_S8T_S8N_S8T:
  y[b,m,n] = sat_i8(round(alpha * sum_k a[b,m,k] * b[b,n,k]))
with a,b int8 [128, 1024, 128], alpha scalar.

Strategy (8 NeuronCores, batch-parallel, 16 batches/core):
 - Host: XOR inputs with 0x80 (biased-uint8 encoding). On-chip transposes move
   raw bytes; the evacuation op subtracts 128 to recover signed values exactly.
 - Per 128x128 input tile: PE transpose in fp8e4-bitcast mode (pure byte
   permutation, output stride-2 in PSUM with zero high bytes).
 - Evacuate PSUM as uint16 (step-1, 2x DVE mode) -> bf16 tiles, minus 128.
 - Main matmuls in bf16 with fp32 PSUM accumulation: bit-exact for int8 data
   (|acc| <= 2^21 < 2^24).
 - Epilogue: single tensor_scalar/activation op: int8 out = rne_sat(alpha*acc),
   which matches jnp.clip(jnp.round(alpha*acc), -128, 127) bit-exactly on HW.
"""

import sys

sys.path.insert(0, "/opt/trn_rl_repo")

import numpy as np

N_CORES = 8
B, M, N, K = 128, 1024, 1024, 128
BPC = B // N_CORES  # batches per core
MT = M // 128  # m tiles per batch
NT = N // 128  # n tiles per batch

_cache = {}


def _build(alpha: float):
    import concourse.bacc as bacc
    import concourse.tile as tile
    import concourse.mybir as mybir
    from concourse.masks import make_identity

    nc = bacc.Bacc("TRN2", target_bir_lowering=False, debug=False)

    a_x = nc.dram_tensor("a_x", [BPC, M, K], mybir.dt.int8, kind="ExternalInput")
    b_x = nc.dram_tensor("b_x", [BPC, N, K], mybir.dt.int8, kind="ExternalInput")
    y = nc.dram_tensor("y", [BPC, M, N], mybir.dt.int8, kind="ExternalOutput")

    fp8 = mybir.dt.float8e4
    u16 = mybir.dt.uint16
    bf16 = mybir.dt.bfloat16
    f32 = mybir.dt.float32
    i8 = mybir.dt.int8

    with tile.TileContext(nc) as tc:
        with (
            tc.tile_pool(name="const", bufs=1) as cpool,
            tc.tile_pool(name="inp", bufs=3) as ipool,
            tc.tile_pool(name="bfp", bufs=2) as bfpool,
            tc.tile_pool(name="outp", bufs=6) as opool,
            tc.tile_pool(name="pst", bufs=2, space="PSUM") as pst,
            tc.tile_pool(name="psmm", bufs=2, space="PSUM") as psmm,
        ):
            ident = cpool.tile([128, 128], fp8)
            make_identity(nc, ident[:])

            for bi in range(BPC):
                a_nat = ipool.tile([128, MT, K], i8, tag="a_nat")
                b_nat = ipool.tile([128, NT, K], i8, tag="b_nat")
                nc.sync.dma_start(
                    out=a_nat[:], in_=a_x[bi].rearrange("(t p) k -> p t k", p=128)
                )
                nc.sync.dma_start(
                    out=b_nat[:], in_=b_x[bi].rearrange("(t p) k -> p t k", p=128)
                )

                # transposes: one PSUM bank per operand (8 tiles x 256B)
                ta = pst.tile([128, MT, 128, 2], fp8, tag="ta")
                tb = pst.tile([128, NT, 128, 2], fp8, tag="tb")
                for t in range(MT):
                    nc.tensor.transpose(
                        ta[:, t, :, 0], a_nat[:, t, :].bitcast(fp8), ident[:]
                    )
                for t in range(NT):
                    nc.tensor.transpose(
                        tb[:, t, :, 0], b_nat[:, t, :].bitcast(fp8), ident[:]
                    )

                # evacuate with -128 (recover signed values); uint16 step-1 reads
                at_bf = bfpool.tile([128, M], bf16, tag="at")
                bt_bf = bfpool.tile([128, N], bf16, tag="bt")
                nc.vector.tensor_scalar(
                    out=at_bf[:],
                    in0=ta[:].bitcast(u16).rearrange("p t k o -> p (t k o)"),
                    scalar1=128.0,
                    scalar2=None,
                    op0=mybir.AluOpType.subtract,
                )
                nc.scalar.activation(
                    out=bt_bf[:],
                    in_=tb[:].bitcast(u16).rearrange("p t k o -> p (t k o)"),
                    func=mybir.ActivationFunctionType.Copy,
                    bias=-128.0,
                    scale=1.0,
                )

                for mt in range(MT):
                    mm = psmm.tile([128, N], f32, tag="mm")
                    for nh in range(2):
                        nc.tensor.matmul(
                            mm[:, nh * 512 : (nh + 1) * 512],
                            at_bf[:, mt * 128 : (mt + 1) * 128],
                            bt_bf[:, nh * 512 : (nh + 1) * 512],
                            start=True,
                            stop=True,
                        )
                    y_sb = opool.tile([128, N], i8, tag="y")
                    if mt % 2 == 0:
                        nc.vector.tensor_scalar(
                            out=y_sb[:],
                            in0=mm[:],
                            scalar1=float(alpha),
                            scalar2=None,
                            op0=mybir.AluOpType.mult,
                        )
                    else:
                        nc.scalar.activation(
                            out=y_sb[:],
                            in_=mm[:],
                            func=mybir.ActivationFunctionType.Copy,
                            scale=float(alpha),
                        )
                    nc.sync.dma_start(
                        out=y[bi, mt * 128 : (mt + 1) * 128, :], in_=y_sb[:]
                    )

    nc.compile()
    return nc


def _get_nc(alpha: float):
    key = float(alpha)
    if key not in _cache:
        _cache[key] = _build(key)
    return _cache[key]


def kernel(a, b, alpha):
    from concourse.bass_utils import run_bass_kernel_spmd

    a = np.asarray(a)
    b = np.asarray(b)
    assert a.shape == (B, M, K) and a.dtype == np.int8
    assert b.shape == (B, N, K) and b.dtype == np.int8

    nc = _get_nc(float(alpha))

    # biased-uint8 encoding: bytes xor 0x80; kernel subtracts 128 on-chip
    ax = (a.view(np.uint8) ^ 0x80).view(np.int8)
    bx = (b.view(np.uint8) ^ 0x80).view(np.int8)

    in_maps = [
        {
            "a_x": np.ascontiguousarray(ax[c * BPC : (c + 1) * BPC]),
            "b_x": np.ascontiguousarray(bx[c * BPC : (c + 1) * BPC]),
        }
        for c in range(N_CORES)
    ]
    res = run_bass_kernel_spmd(nc, in_maps, list(range(N_CORES)))
    out = np.concatenate([r["y"] for r in res.results], axis=0)
    return out.astype(np.int8)


# revision 3
# speedup vs baseline: 1.0007x; 1.0007x over previous
"""
Trainium2 Bass kernel for nn_BMM_S8T_S8N_S8T:
  y[b,m,n] = sat_i8(round(alpha * sum_k a[b,m,k] * b[b,n,k]))
with a,b int8 [128, 1024, 128], alpha scalar.

Strategy (8 NeuronCores, batch-parallel, 16 batches/core):
 - Host: XOR inputs with 0x80 (biased-uint8 encoding). On-chip transposes move
   raw bytes; the evacuation op subtracts 128 to recover signed values exactly.
 - Per 128x128 input tile: PE transpose in fp8e4-bitcast mode (pure byte
   permutation, output stride-2 in PSUM with zero high bytes). Transposes are
   interleaved with the main matmuls to keep the PE HAM clock-gate warm.
 - Evacuate PSUM as uint16 -> bf16 tiles, minus 128 (DVE/ACT split).
 - Main matmuls in bf16 with fp32 PSUM accumulation: bit-exact for int8 data
   (|acc| <= 2^21 < 2^24).
 - Epilogue: single tensor_scalar/activation op: int8 out = rne_sat(alpha*acc),
   which matches jnp.clip(jnp.round(alpha*acc), -128, 127) bit-exactly on HW.
 - DMA is spread across both HWDGE queues (sync + scalar) - a single queue
   caps at ~200 GB/s.
"""

import sys

sys.path.insert(0, "/opt/trn_rl_repo")

import numpy as np

N_CORES = 8
B, M, N, K = 128, 1024, 1024, 128
BPC = B // N_CORES  # batches per core
MT = M // 128  # m tiles per batch
NT = N // 128  # n tiles per batch

_cache = {}


def _build(alpha: float):
    import concourse.bacc as bacc
    import concourse.tile as tile
    import concourse.mybir as mybir
    from concourse.masks import make_identity

    nc = bacc.Bacc("TRN2", target_bir_lowering=False, debug=False)

    a_x = nc.dram_tensor("a_x", [BPC, M, K], mybir.dt.int8, kind="ExternalInput")
    b_x = nc.dram_tensor("b_x", [BPC, N, K], mybir.dt.int8, kind="ExternalInput")
    y = nc.dram_tensor("y", [BPC, M, N], mybir.dt.int8, kind="ExternalOutput")

    fp8 = mybir.dt.float8e4
    u16 = mybir.dt.uint16
    bf16 = mybir.dt.bfloat16
    f32 = mybir.dt.float32
    i8 = mybir.dt.int8

    with tile.TileContext(nc) as tc:
        with (
            tc.tile_pool(name="const", bufs=1) as cpool,
            tc.tile_pool(name="inp", bufs=3) as ipool,
            tc.tile_pool(name="bfp", bufs=2) as bfpool,
            tc.tile_pool(name="outp", bufs=6) as opool,
            tc.tile_pool(name="pst", bufs=2, space="PSUM") as pst,
            tc.tile_pool(name="psmm", bufs=2, space="PSUM") as psmm,
        ):
            ident = cpool.tile([128, 128], fp8)
            make_identity(nc, ident[:])

            # per-batch state created one batch ahead (software pipeline)
            nat = [None] * BPC  # (a_nat, b_nat)
            tps = [None] * BPC  # (ta, tb)

            def load_batch(bi):
                a_nat = ipool.tile([128, MT, K], i8, tag="a_nat")
                b_nat = ipool.tile([128, NT, K], i8, tag="b_nat")
                # loads on the scalar HWDGE queue
                nc.scalar.dma_start(
                    out=a_nat[:], in_=a_x[bi].rearrange("(t p) k -> p t k", p=128)
                )
                nc.scalar.dma_start(
                    out=b_nat[:], in_=b_x[bi].rearrange("(t p) k -> p t k", p=128)
                )
                nat[bi] = (a_nat, b_nat)

            def transpose_pair(bi, idx):
                # transpose tile-pair idx (a-tile idx and b-tile idx) of batch bi
                a_nat, b_nat = nat[bi]
                if idx == 0:
                    ta = pst.tile([128, MT, 128, 2], fp8, tag="ta")
                    tb = pst.tile([128, NT, 128, 2], fp8, tag="tb")
                    tps[bi] = (ta, tb)
                ta, tb = tps[bi]
                nc.tensor.transpose(
                    ta[:, idx, :, 0], a_nat[:, idx, :].bitcast(fp8), ident[:]
                )
                nc.tensor.transpose(
                    tb[:, idx, :, 0], b_nat[:, idx, :].bitcast(fp8), ident[:]
                )

            def evac_batch(bi):
                ta, tb = tps[bi]
                at_bf = bfpool.tile([128, M], bf16, tag="at")
                bt_bf = bfpool.tile([128, N], bf16, tag="bt")
                nc.vector.tensor_scalar(
                    out=at_bf[:],
                    in0=ta[:].bitcast(u16).rearrange("p t k o -> p (t k o)"),
                    scalar1=128.0,
                    scalar2=None,
                    op0=mybir.AluOpType.subtract,
                )
                nc.scalar.activation(
                    out=bt_bf[:],
                    in_=tb[:].bitcast(u16).rearrange("p t k o -> p (t k o)"),
                    func=mybir.ActivationFunctionType.Copy,
                    bias=-128.0,
                    scale=1.0,
                )
                return at_bf, bt_bf

            # prologue: batch 0 loads + transposes
            load_batch(0)
            for t in range(MT):
                transpose_pair(0, t)

            for bi in range(BPC):
                at_bf, bt_bf = evac_batch(bi)
                if bi + 1 < BPC:
                    load_batch(bi + 1)

                for mt in range(MT):
                    mm = psmm.tile([128, N], f32, tag="mm")
                    for nh in range(2):
                        nc.tensor.matmul(
                            mm[:, nh * 512 : (nh + 1) * 512],
                            at_bf[:, mt * 128 : (mt + 1) * 128],
                            bt_bf[:, nh * 512 : (nh + 1) * 512],
                            start=True,
                            stop=True,
                        )
                    # interleave next batch's transposes between MM pairs
                    # to keep PE activity dense (HAM warm)
                    if bi + 1 < BPC:
                        transpose_pair(bi + 1, mt)

                    y_sb = opool.tile([128, N], i8, tag="y")
                    # DVE/ACT split; alternate 4/4 and 3/5 to balance
                    use_dve = (mt % 2 == 0) if bi % 2 == 0 else (mt % 2 == 0 and mt != 6)
                    if use_dve:
                        nc.vector.tensor_scalar(
                            out=y_sb[:],
                            in0=mm[:],
                            scalar1=float(alpha),
                            scalar2=None,
                            op0=mybir.AluOpType.mult,
                        )
                    else:
                        nc.scalar.activation(
                            out=y_sb[:],
                            in_=mm[:],
                            func=mybir.ActivationFunctionType.Copy,
                            scale=float(alpha),
                        )
                    # stores alternate between the two HWDGE queues
                    store_eng = nc.sync if mt % 2 == 0 else nc.scalar
                    store_eng.dma_start(
                        out=y[bi, mt * 128 : (mt + 1) * 128, :], in_=y_sb[:]
                    )

    nc.compile()
    return nc


def _get_nc(alpha: float):
    key = float(alpha)
    if key not in _cache:
        _cache[key] = _build(key)
    return _cache[key]


def kernel(a, b, alpha):
    from concourse.bass_utils import run_bass_kernel_spmd

    a = np.asarray(a)
    b = np.asarray(b)
    assert a.shape == (B, M, K) and a.dtype == np.int8
    assert b.shape == (B, N, K) and b.dtype == np.int8

    nc = _get_nc(float(alpha))

    # biased-uint8 encoding: bytes xor 0x80; kernel subtracts 128 on-chip
    ax = (a.view(np.uint8) ^ 0x80).view(np.int8)
    bx = (b.view(np.uint8) ^ 0x80).view(np.int8)

    in_maps = [
        {
            "a_x": np.ascontiguousarray(ax[c * BPC : (c + 1) * BPC]),
            "b_x": np.ascontiguousarray(bx[c * BPC : (c + 1) * BPC]),
        }
        for c in range(N_CORES)
    ]
    res = run_bass_kernel_spmd(nc, in_maps, list(range(N_CORES)))
    out = np.concatenate([r["y"] for r in res.results], axis=0)
    return out.astype(np.int8)


# revision 4
# speedup vs baseline: 1.1204x; 1.1196x over previous
"""
Trainium2 Bass kernel for nn_BMM_S8T_S8N_S8T:
  y[b,m,n] = sat_i8(round(alpha * sum_k a[b,m,k] * b[b,n,k]))
with a,b int8 [128, 1024, 128], alpha scalar.

Strategy (8 NeuronCores, batch-parallel, 16 batches/core):
 - Host: XOR inputs with 0x80 (biased-uint8 encoding). On-chip transposes move
   raw bytes; the evacuation op subtracts 128 to recover signed values exactly.
 - Per 128x128 input tile: PE transpose in fp8e4-bitcast mode (pure byte
   permutation, output stride-2 in PSUM with zero high bytes). Transposes are
   interleaved with the main matmuls to keep the PE HAM clock-gate warm.
 - Evacuate PSUM as uint16 -> bf16 tiles, minus 128 (DVE/ACT split).
 - Main matmuls in bf16 with fp32 PSUM accumulation: bit-exact for int8 data
   (|acc| <= 2^21 < 2^24).
 - Epilogue: single tensor_scalar/activation op: int8 out = rne_sat(alpha*acc),
   which matches jnp.clip(jnp.round(alpha*acc), -128, 127) bit-exactly on HW.
 - DMA is spread across both HWDGE queues (sync + scalar) - a single queue
   caps at ~200 GB/s.
"""

import sys

sys.path.insert(0, "/opt/trn_rl_repo")

import numpy as np

N_CORES = 8
B, M, N, K = 128, 1024, 1024, 128
BPC = B // N_CORES  # batches per core
MT = M // 128  # m tiles per batch
NT = N // 128  # n tiles per batch

_cache = {}


def _build(alpha: float):
    import concourse.bacc as bacc
    import concourse.tile as tile
    import concourse.mybir as mybir
    from concourse.masks import make_identity

    nc = bacc.Bacc("TRN2", target_bir_lowering=False, debug=False)

    a_x = nc.dram_tensor("a_x", [BPC, M, K], mybir.dt.int8, kind="ExternalInput")
    b_x = nc.dram_tensor("b_x", [BPC, N, K], mybir.dt.int8, kind="ExternalInput")
    y = nc.dram_tensor("y", [BPC, M, N], mybir.dt.int8, kind="ExternalOutput")

    fp8 = mybir.dt.float8e4
    u16 = mybir.dt.uint16
    bf16 = mybir.dt.bfloat16
    f32 = mybir.dt.float32
    i8 = mybir.dt.int8

    with tile.TileContext(nc) as tc:
        with (
            tc.tile_pool(name="const", bufs=1) as cpool,
            tc.tile_pool(name="inp", bufs=3) as ipool,
            tc.tile_pool(name="bfp", bufs=2) as bfpool,
            tc.tile_pool(name="outp", bufs=6) as opool,
            tc.tile_pool(name="pst", bufs=1, space="PSUM") as pst,
            tc.tile_pool(name="psmm", bufs=3, space="PSUM") as psmm,
        ):
            ident = cpool.tile([128, 128], fp8)
            make_identity(nc, ident[:])

            # per-batch state created one batch ahead (software pipeline)
            nat = [None] * BPC  # (a_nat, b_nat)
            tps = [None] * BPC  # (ta, tb)

            def load_batch(bi):
                a_nat = ipool.tile([128, MT, K], i8, tag="a_nat")
                b_nat = ipool.tile([128, NT, K], i8, tag="b_nat")
                # loads on the scalar HWDGE queue
                nc.sync.dma_start(
                    out=a_nat[:], in_=a_x[bi].rearrange("(t p) k -> p t k", p=128)
                )
                nc.sync.dma_start(
                    out=b_nat[:], in_=b_x[bi].rearrange("(t p) k -> p t k", p=128)
                )
                nat[bi] = (a_nat, b_nat)

            def transpose_pair(bi, idx):
                # transpose tile-pair idx (a-tile idx and b-tile idx) of batch bi
                a_nat, b_nat = nat[bi]
                if idx == 0:
                    ta = pst.tile([128, MT, 128, 2], fp8, tag="ta")
                    tb = pst.tile([128, NT, 128, 2], fp8, tag="tb")
                    tps[bi] = (ta, tb)
                ta, tb = tps[bi]
                nc.tensor.transpose(
                    ta[:, idx, :, 0], a_nat[:, idx, :].bitcast(fp8), ident[:]
                )
                nc.tensor.transpose(
                    tb[:, idx, :, 0], b_nat[:, idx, :].bitcast(fp8), ident[:]
                )

            def evac_batch(bi):
                # 2-stage on DVE: u16 copy PSUM->SBUF (2x mode), then
                # subtract-128 u16->bf16 SBUF->SBUF (4x mode)
                ta, tb = tps[bi]
                at_bf = bfpool.tile([128, M], bf16, tag="at")
                bt_bf = bfpool.tile([128, N], bf16, tag="bt")
                for ps_t, out_bf, tag in ((ta, at_bf, "rawa"), (tb, bt_bf, "rawb")):
                    raw = bfpool.tile([128, 1024], u16, tag=tag)
                    nc.vector.tensor_copy(
                        out=raw[:],
                        in_=ps_t[:].bitcast(u16).rearrange("p t k o -> p (t k o)"),
                    )
                    nc.vector.tensor_scalar(
                        out=out_bf[:],
                        in0=raw[:],
                        scalar1=128.0,
                        scalar2=None,
                        op0=mybir.AluOpType.subtract,
                    )
                return at_bf, bt_bf

            # prologue: batch 0 loads + transposes
            load_batch(0)
            for t in range(MT):
                transpose_pair(0, t)

            for bi in range(BPC):
                at_bf, bt_bf = evac_batch(bi)
                if bi + 1 < BPC:
                    load_batch(bi + 1)

                for mt in range(MT):
                    mm = psmm.tile([128, N], f32, tag="mm")
                    for nh in range(2):
                        nc.tensor.matmul(
                            mm[:, nh * 512 : (nh + 1) * 512],
                            at_bf[:, mt * 128 : (mt + 1) * 128],
                            bt_bf[:, nh * 512 : (nh + 1) * 512],
                            start=True,
                            stop=True,
                        )
                    # next batch's transposes, grouped mid-batch
                    if bi + 1 < BPC and mt == 1:
                        for t in range(MT):
                            transpose_pair(bi + 1, t)

                    y_sb = opool.tile([128, N], i8, tag="y")
                    # DVE/ACT split: DVE 3/batch (it also does evacs), ACT 5
                    use_dve = mt in (0, 3, 6)
                    if use_dve:
                        nc.vector.tensor_scalar(
                            out=y_sb[:],
                            in0=mm[:],
                            scalar1=float(alpha),
                            scalar2=None,
                            op0=mybir.AluOpType.mult,
                        )
                    else:
                        nc.scalar.activation(
                            out=y_sb[:],
                            in_=mm[:],
                            func=mybir.ActivationFunctionType.Copy,
                            scale=float(alpha),
                        )
                    # stores alternate between the two HWDGE queues
                    store_eng = nc.sync if mt % 2 == 0 else nc.scalar
                    store_eng.dma_start(
                        out=y[bi, mt * 128 : (mt + 1) * 128, :], in_=y_sb[:]
                    )

    nc.compile()
    return nc


def _get_nc(alpha: float):
    key = float(alpha)
    if key not in _cache:
        _cache[key] = _build(key)
    return _cache[key]


def kernel(a, b, alpha):
    from concourse.bass_utils import run_bass_kernel_spmd

    a = np.asarray(a)
    b = np.asarray(b)
    assert a.shape == (B, M, K) and a.dtype == np.int8
    assert b.shape == (B, N, K) and b.dtype == np.int8

    nc = _get_nc(float(alpha))

    # biased-uint8 encoding: bytes xor 0x80; kernel subtracts 128 on-chip
    ax = (a.view(np.uint8) ^ 0x80).view(np.int8)
    bx = (b.view(np.uint8) ^ 0x80).view(np.int8)

    in_maps = [
        {
            "a_x": np.ascontiguousarray(ax[c * BPC : (c + 1) * BPC]),
            "b_x": np.ascontiguousarray(bx[c * BPC : (c + 1) * BPC]),
        }
        for c in range(N_CORES)
    ]
    res = run_bass_kernel_spmd(nc, in_maps, list(range(N_CORES)))
    out = np.concatenate([r["y"] for r in res.results], axis=0)
    return out.astype(np.int8)
